# revision 17
# baseline (speedup 1.0000x reference)
"""BiMamba layer on 8 TRN2 NeuronCores.

Sharding: 8 cores = 4 (dir,batch) pairs x 2 halves of d_inner; host flips
the sequence for the backward direction, transposes to [channel, token]
layout, and sums the 4 partial outputs per batch + residual at the end.

Per-core pipeline (all on-chip, [channel, token] layout):
  LN (stats via ones-matmul, rstd = Exp(-0.5*Ln(var+eps)))
  -> in_proj (f32r matmuls; LN gain/bias folded into weights on host)
  -> depthwise causal conv (tensor_scalar + shifted scalar_tensor_tensor)
  -> silu via Sigmoid+mult -> xproj/dtproj (bf16 matmuls)
  -> softplus = Ln(1+Exp(x)) (same ACT table as the scan exps)
  -> selective scan: A[d,n] = -n  =>  dA_n = Exp(-n*dt) with immediate ACT
     scale; recurrence via DVE tensor_tensor_scan per n; y = sum_n C_n*h_n
  -> y = (D*u + ysc) * silu(z) -> fused (out_proj @ fuse) matmul (f32r).

Channel permutation on host makes the SPMD program uniform: each core's
512 scan channels are always xc rows 0..511.
"""
import sys
sys.path.insert(0, '/opt/trn_rl_repo')
import numpy as np
import ml_dtypes
from contextlib import ExitStack

import concourse.bass as bass
import concourse.tile as tile
from concourse import bacc, mybir
from concourse.bass_utils import run_bass_kernel_spmd

AF = mybir.ActivationFunctionType
OP = mybir.AluOpType
F32, BF16, F32R = mybir.dt.float32, mybir.dt.bfloat16, mybir.dt.float32r
BF = ml_dtypes.bfloat16

DIM, DSTATE, DCONV, DINNER, DTRANK, B, L = 512, 16, 4, 1024, 32, 2, 2048
HALF = DINNER // 2
P = 128
NT = L // 512
KD = DIM // P               # 4 k-tiles over D
MI = (DINNER + HALF) // P   # 12 in_proj M-tiles (8 xc + 4 z)
MX = DINNER // P            # 8 xc tiles
MH = HALF // P              # 4 scan-channel tiles
EPS = 1e-5

_CACHE = {}


def _build(trace_sim=False):
    nc = bacc.Bacc("TRN2", target_bir_lowering=False, debug=False,
                   num_devices=8)
    dram = {}
    def din(name, shape, dt):
        dram[name] = nc.dram_tensor(name, shape, dt, kind="ExternalInput").ap()
    din("xT", [DIM, L], F32R)
    din("inw", [DIM, P * MI], F32R)
    din("r1w", [1, 2 * P * MI], BF16)
    din("convw", [DINNER, DCONV], F32)
    din("cbrow", [1, DINNER], BF16)
    din("xpw", [DINNER, 64], BF16)
    din("dtpw", [DTRANK, HALF], BF16)
    din("dtb", [P, MH], F32)
    din("dvec", [P, MH], F32)
    din("weff", [HALF, DIM], BF16)
    din("ones", [P, 1], F32R)
    din("ident", [P, P], BF16)
    outT = nc.dram_tensor("outT", [DIM, L], F32, kind="ExternalOutput").ap()
    bcrows = nc.dram_tensor("bcrows", [2 * DSTATE, L], BF16).ap()
    stats = nc.dram_tensor("stats", [1, L], BF16).ap()

    with tile.TileContext(nc, trace_sim=trace_sim) as tc, ExitStack() as ctx:
        sb = ctx.enter_context(tc.tile_pool(name="sb", bufs=1))
        tf = ctx.enter_context(tc.tile_pool(name="tf", bufs=2))
        ppA = tc.alloc_tile_pool(name="ppA", bufs=2, space="PSUM")

        # ---- weights ----
        inw = [sb.tile([P, P * MI], F32R, tag=f"w{k}", name=f"w{k}") for k in range(KD)]
        for k in range(KD):
            nc.sync.dma_start(inw[k][:], dram["inw"][k * P:(k + 1) * P, :])
        r1w = sb.tile([1, 2 * P * MI], BF16, tag="r1w", name="r1w")
        nc.sync.dma_start(r1w[:], dram["r1w"][:])
        convw = [sb.tile([P, DCONV], F32, tag=f"cw{j}", name=f"cw{j}") for j in range(MX)]
        for j in range(MX):
            nc.sync.dma_start(convw[j][:], dram["convw"][j * P:(j + 1) * P, :])
        cbrow = sb.tile([1, DINNER], BF16, tag="cbrow", name="cbrow")
        nc.sync.dma_start(cbrow[:], dram["cbrow"][:])
        xpw = [sb.tile([P, 64], BF16, tag=f"xpw{k}", name=f"xpw{k}") for k in range(MX)]
        for k in range(MX):
            nc.sync.dma_start(xpw[k][:], dram["xpw"][k * P:(k + 1) * P, :])
        dtpw = sb.tile([DTRANK, HALF], BF16, tag="dtpw", name="dtpw")
        nc.sync.dma_start(dtpw[:], dram["dtpw"][:])
        dtb = sb.tile([P, MH], F32, tag="dtb", name="dtb")
        nc.sync.dma_start(dtb[:], dram["dtb"][:])
        dvec = sb.tile([P, MH], F32, tag="dvec", name="dvec")
        nc.sync.dma_start(dvec[:], dram["dvec"][:])
        weff = [sb.tile([P, DIM], BF16, tag=f"wef{k}", name=f"wef{k}") for k in range(MH)]
        for k in range(MH):
            nc.sync.dma_start(weff[k][:], dram["weff"][k * P:(k + 1) * P, :])
        ones = sb.tile([P, 1], F32R, tag="ones", name="ones")
        nc.sync.dma_start(ones[:], dram["ones"][:])
        ident = sb.tile([P, P], BF16, tag="ident", name="ident")
        nc.sync.dma_start(ident[:], dram["ident"][:])
        ceps = sb.tile([1, 1], F32, tag="ceps", name="ceps")
        nc.vector.memset(ceps[:], EPS)
        cone = sb.tile([P, 1], F32, tag="cone", name="cone")
        nc.vector.memset(cone[:], 1.0)
        ones16 = sb.tile([P, 1], BF16, tag="ones16", name="ones16")
        nc.vector.memset(ones16[:], 1.0)

        # ---- phase A: LN stats on raw xT; mean/bias folded into in_proj via
        #      rank-1 matmul rows; rstd folded into the PSUM evacuation ----
        pssum = ppA.tile([1, L], F32, tag="pa", name="st0")
        pssq = ppA.tile([1, L], F32, tag="pa", name="st1")
        h0 = []
        for k in range(KD):
            xtk = sb.tile([P, L], F32R, tag=f"g{k}", name=f"g{k}")
            nc.sync.dma_start(xtk[:], dram["xT"][k * P:(k + 1) * P, :])
            h0.append(xtk)
            xsq = sb.tile([P, L], BF16, tag=f"m{k % 2}", name=f"m{k % 2}")
            nc.scalar.activation(xsq[:], xtk[:], AF.Square)
            for c in range(NT):
                sl = slice(c * 512, (c + 1) * 512)
                nc.tensor.matmul(pssum[:, sl], ones[:],
                                 xtk[:, sl],
                                 start=(k == 0), stop=(k == KD - 1))
                nc.tensor.matmul(pssq[:, sl], ones16[:],
                                 xsq[:, sl],
                                 start=(k == 0), stop=(k == KD - 1))
        mu = sb.tile([1, L], F32, tag="s0", name="mu")
        m2 = sb.tile([1, L], F32, tag="s1", name="m2")
        nc.scalar.activation(mu[:], pssum[:], AF.Copy, scale=1.0 / DIM)
        nc.scalar.activation(m2[:], pssq[:], AF.Copy, scale=1.0 / DIM)
        mu2 = ppA.tile([1, L], F32, tag="pa", name="mu2")
        nc.vector.tensor_tensor(mu2[:], mu[:], mu[:], OP.mult)
        var = ppA.tile([1, L], F32, tag="pa", name="var")
        nc.vector.tensor_tensor(var[:], m2[:], mu2[:], OP.subtract)
        lnv = ppA.tile([1, L], F32, tag="pa", name="lnv")
        nc.scalar.activation(lnv[:], var[:], AF.Ln, bias=ceps[:])
        # r1x rows: [mu; std] — rank-1 rhs so (W@x - w1*mu + inb*std)*rstd
        # reconstructs W@LN(x) + inb after the rstd evac multiply
        mu16 = sb.tile([1, L], BF16, tag="s1", name="mu16")
        nc.scalar.activation(mu16[:], mu[:], AF.Copy)
        std16 = sb.tile([1, L], BF16, tag="s2", name="std16")
        nc.scalar.activation(std16[:], lnv[:], AF.Exp, scale=0.5)
        rstd16 = sb.tile([1, L], BF16, tag="s0", name="rstd16")
        nc.scalar.activation(rstd16[:], lnv[:], AF.Exp, scale=-0.5)
        nc.sync.dma_start(stats[0:1, :], rstd16[:])
        rstd_b = sb.tile([P, L], BF16, tag="m1", name="m1")
        nc.sync.dma_start(rstd_b[:], stats[0:1, :].broadcast_to([P, L]))

        # ---- phase B: in_proj (raw x; rank-1 rows fold LN mean + bias) ----
        ppA.release()
        ppB = tc.alloc_tile_pool(name="ppB", bufs=4, space="PSUM")
        ppC = tc.alloc_tile_pool(name="ppC", bufs=1, space="PSUM")
        xc = [sb.tile([P, L], BF16, tag=f"x{j}", name=f"x{j}") for j in range(MX)]
        z = [sb.tile([P, L], BF16, tag=f"z{j}", name=f"z{j}") for j in range(MI - MX)]
        for m in range(MI):
            dest = xc[m] if m < MX else z[m - MX]
            for c in range(NT):
                sl = slice(c * 512, (c + 1) * 512)
                pmm = ppB.tile([P, 512], F32, tag="mmb", name="mmb")
                for k in range(KD):
                    nc.tensor.matmul(pmm[:], inw[k][:, m * P:(m + 1) * P],
                                     h0[k][:, sl],
                                     start=(k == 0), stop=False)
                nc.tensor.matmul(pmm[:], r1w[0:1, m * P:(m + 1) * P],
                                 mu16[:, sl], start=False, stop=False)
                nc.tensor.matmul(pmm[:], r1w[0:1, P * MI + m * P:P * MI + (m + 1) * P],
                                 std16[:, sl], start=False, stop=True)
                nc.vector.tensor_tensor(dest[:, sl], pmm[:], rstd_b[:, sl],
                                        OP.mult)

        # ---- phase C: depthwise conv on PE (diag-weight matmuls, shifted
        #      rhs; bias via rank-1); silu straight off PSUM -> u ----
        ones512 = sb.tile([1, 512], BF16, tag="o512", name="o512")
        nc.vector.memset(ones512[:], 1.0)
        u = []
        for j in range(MX):
            dj = []
            for k in range(DCONV):
                dk = sb.tile([P, P], BF16, tag=f"dw{k}_{j % 2}", name=f"dw{k}_{j % 2}")
                nc.vector.tensor_scalar_mul(dk[:], ident[:], convw[j][:, k:k + 1])
                dj.append(dk)
            uj = sb.tile([P, L], BF16, tag=f"x{j}", name=f"x{j}")
            for c in range(NT):
                sl = slice(c * 512, (c + 1) * 512)
                pcv = ppB.tile([P, 512], F32, tag="mmb", name="cv")
                nc.tensor.matmul(pcv[:], dj[3][:], xc[j][:, sl],
                                 start=True, stop=False)
                for k in range(3):
                    off = 3 - k
                    lo = c * 512 - off
                    if lo < 0:
                        nc.tensor.matmul(pcv[:, off:], dj[k][:],
                                         xc[j][:, 0:512 - off],
                                         start=False, stop=False)
                    else:
                        nc.tensor.matmul(pcv[:], dj[k][:],
                                         xc[j][:, lo:lo + 512],
                                         start=False, stop=False)
                nc.tensor.matmul(pcv[:], cbrow[0:1, j * P:(j + 1) * P],
                                 ones512[:], start=False, stop=True)
                sg = tf.tile([P, 512], BF16, tag="tf", name="sg")
                nc.scalar.activation(sg[:], pcv[:], AF.Sigmoid)
                nc.vector.tensor_tensor(uj[:, sl], pcv[:], sg[:], OP.mult)
            u.append(uj)

        # ---- phase D: xproj, dtproj, softplus, dtu, D*u, silu(z) ----
        px = ppC.tile([64, L], F32, tag="big", name="px")
        for c in range(NT):
            sl = slice(c * 512, (c + 1) * 512)
            for k in range(MX):
                nc.tensor.matmul(px[:, sl], xpw[k][:], u[k][:, sl],
                                 start=(k == 0), stop=(k == MX - 1))
        dbl = sb.tile([64, L], BF16, tag="dbl", name="dbl")
        for c in range(NT):
            sl = slice(c * 512, (c + 1) * 512)
            nc.scalar.activation(dbl[:, sl], px[:, sl], AF.Copy)
        nc.sync.dma_start(bcrows[:], dbl[DTRANK:64, :])

        dt = []
        dtu = []
        yD = []
        for m in range(MH):
            pd = ppC.tile([P, L], F32, tag="big", name="pd")
            for c in range(NT):
                sl = slice(c * 512, (c + 1) * 512)
                nc.tensor.matmul(pd[:, sl], dtpw[:, m * P:(m + 1) * P],
                                 dbl[0:DTRANK, sl], start=True, stop=True)
            dtm = sb.tile([P, L], BF16, tag=f"f{m}", name=f"f{m}")
            nc.scalar.activation(dtm[:], pd[:], AF.Exp, bias=dtb[:, m:m + 1])
            nc.scalar.activation(dtm[:], dtm[:], AF.Ln, bias=cone[:])
            dt.append(dtm)
            dtum = sb.tile([P, L], BF16, tag=("m0", "m1", "d2", "d3")[m], name="dtu")
            nc.vector.tensor_tensor(dtum[:], dtm[:], u[m][:], OP.mult)
            dtu.append(dtum)
            yDm = sb.tile([P, L], BF16, tag=f"yd{m}", name=f"yd{m}")
            nc.vector.tensor_scalar_mul(yDm[:], u[m][:], dvec[:, m:m + 1])
            yD.append(yDm)
        zs = []
        for j in range(MH):
            sgz = tf.tile([P, L], BF16, tag="tf", name="tf")
            nc.scalar.activation(sgz[:], z[j][:], AF.Sigmoid)
            zsj = sb.tile([P, L], BF16, tag=f"x{MH + j}", name=f"x{MH + j}")
            nc.vector.tensor_tensor(zsj[:], z[j][:], sgz[:], OP.mult)
            zs.append(zsj)

        # ---- phase E: 16 scan passes; ysc accumulated in PSUM via
        #      identity-matmul on PE (f32 accumulation, no DVE adds) ----
        ppC.release()
        ppB.release()
        ppE = tc.alloc_tile_pool(name="ppE", bufs=2, space="PSUM")
        y = []
        for mhalf in range(2):
            yp = [ppE.tile([P, L], F32, tag="ysp", name="ysp") for _ in range(2)]
            for n in range(1, DSTATE + 1):
                bn = sb.tile([P, L], BF16, tag=f"x{n % 2}", name="bn")
                cn = sb.tile([P, L], BF16, tag=f"x{2 + n % 2}", name="cn")
                nc.sync.dma_start(bn[:], bcrows[n - 1:n, :].broadcast_to([P, L]))
                nc.sync.dma_start(cn[:], bcrows[DSTATE + n - 1:DSTATE + n, :]
                                  .broadcast_to([P, L]))
                for j in range(2):
                    m = mhalf * 2 + j
                    it = n * 2 + j
                    dA = sb.tile([P, L], BF16, tag=f"w{it % 2}", name="dA")
                    nc.scalar.activation(dA[:], dt[m][:], AF.Exp, scale=float(-n))
                    dBu = sb.tile([P, L], BF16, tag=f"w{2 + it % 2}", name="dBu")
                    nc.vector.tensor_tensor(dBu[:], dtu[m][:], bn[:], OP.mult)
                    h = sb.tile([P, L], BF16, tag=f"z{it % 2}", name="h")
                    nc.vector.tensor_tensor_scan(h[:], dA[:], dBu[:], 0.0,
                                                 OP.mult, OP.add)
                    hc = sb.tile([P, L], BF16, tag=f"z{2 + it % 2}", name="hc")
                    hc_eng = nc.gpsimd if n % 2 == 0 else nc.vector
                    hc_eng.tensor_tensor(hc[:], h[:], cn[:], OP.mult)
                    for c in range(NT):
                        sl = slice(c * 512, (c + 1) * 512)
                        nc.tensor.matmul(yp[j][:, sl], ident[:], hc[:, sl],
                                         start=(n == 1), stop=(n == DSTATE))
            for j in range(2):
                m = mhalf * 2 + j
                ym = sb.tile([P, L], BF16, tag=f"g{m}", name=f"g{m}")
                nc.vector.tensor_tensor(ym[:], yD[m][:], yp[j][:], OP.add)
                nc.vector.tensor_tensor(ym[:], ym[:], zs[m][:], OP.mult)
                y.append(ym)

        # ---- phase F: y = (D*u + ysc) * zs; out_proj ----
        ppE.release()
        ppF = tc.alloc_tile_pool(name="ppF", bufs=2, space="PSUM")
        for half in range(2):
            po = [ppF.tile([P, L], F32, tag="po", name="po") for _ in range(2)]
            for k in range(MH):
                for j in range(2):
                    mo = half * 2 + j
                    for c in range(NT):
                        sl = slice(c * 512, (c + 1) * 512)
                        nc.tensor.matmul(po[j][:, sl],
                                         weff[k][:, mo * P:(mo + 1) * P],
                                         y[k][:, sl],
                                         start=(k == 0), stop=(k == MH - 1))
            for j in range(2):
                mo = half * 2 + j
                for c in range(NT):
                    sl = slice(c * 512, (c + 1) * 512)
                    ev = tf.tile([P, 512], F32, tag="tf", name="ev")
                    if (j * NT + c) % 2 == 0:
                        nc.scalar.activation(ev[:], po[j][:, sl], AF.Copy)
                    else:
                        nc.vector.tensor_copy(ev[:], po[j][:, sl])
                    nc.sync.dma_start(outT[mo * P:(mo + 1) * P, sl], ev[:])
        ppF.release()
    nc.compile()
    return nc


def _host_prep(inputs):
    f32 = np.float32
    x = np.asarray(inputs["x"], f32)
    ln_g = np.asarray(inputs["ln_g"], f32); ln_b = np.asarray(inputs["ln_b"], f32)
    in_w = np.asarray(inputs["in_w"], f32)
    conv_w = np.asarray(inputs["conv_w"], f32); conv_b = np.asarray(inputs["conv_b"], f32)
    xproj_w = np.asarray(inputs["xproj_w"], f32); dtproj_w = np.asarray(inputs["dtproj_w"], f32)
    dt_bias = np.asarray(inputs["dt_bias"], f32)
    D = np.asarray(inputs["D"], f32)
    out_w = np.asarray(inputs["out_w"], f32)
    fuse_w = np.asarray(inputs["fuse_w"], f32)

    maps = []
    for p in range(4):
        dir_, b = p // 2, p % 2
        W = in_w[dir_] * ln_g[None, :]          # [2*Di, D], LN gain folded
        in_bias_full = in_w[dir_] @ ln_b        # LN bias folded
        Weff_out = fuse_w[:, dir_ * DIM:(dir_ + 1) * DIM] @ out_w[dir_]
        xb = x[b] if dir_ == 0 else x[b, ::-1]
        for half in range(2):
            sl = slice(half * HALF, (half + 1) * HALF)
            # permute xc channels so this core's scan channels are rows 0..511
            perm = np.concatenate([np.arange(half * HALF, (half + 1) * HALF),
                                   np.arange((1 - half) * HALF, (2 - half) * HALF)])
            rows = np.concatenate([perm, DINNER + np.arange(half * HALF, (half + 1) * HALF)])
            Wr = W[rows]
            m = dict(
                xT=np.ascontiguousarray(xb.T),
                inw=np.ascontiguousarray(Wr.T),
                r1w=np.ascontiguousarray(np.concatenate(
                    [-Wr.sum(axis=1), in_bias_full[rows]]).reshape(1, -1).astype(BF)),
                convw=np.ascontiguousarray(conv_w[dir_][perm]),
                cbrow=np.ascontiguousarray(
                    conv_b[dir_][perm].reshape(1, DINNER).astype(BF)),
                xpw=np.ascontiguousarray(xproj_w[dir_][:, perm].T.astype(BF)),
                dtpw=np.ascontiguousarray(dtproj_w[dir_, sl].T.astype(BF)),
                dtb=np.ascontiguousarray(dt_bias[dir_, sl].reshape(MH, P).T),
                dvec=np.ascontiguousarray(D[dir_, sl].reshape(MH, P).T),
                weff=np.ascontiguousarray(Weff_out[:, sl].T.astype(BF)),
                ones=np.ones((P, 1), np.float32),
                ident=np.eye(P, dtype=np.float32).astype(BF),
            )
            maps.append(m)
    return maps


def kernel(**inputs):
    if "nc" not in _CACHE:
        _CACHE["nc"] = _build()
    nc = _CACHE["nc"]
    maps = _host_prep(inputs)
    res = run_bass_kernel_spmd(nc, maps, list(range(8)))
    x = np.asarray(inputs["x"], np.float32)
    fuse_b = np.asarray(inputs["fuse_b"], np.float32)
    out = x + fuse_b[None, None, :]
    for p in range(4):
        dir_, b = p // 2, p % 2
        for half in range(2):
            pt = np.asarray(res.results[p * 2 + half]["outT"], np.float32).T
            if dir_ == 1:
                pt = pt[::-1]
            out[b] += pt
    return out.astype(np.float32)



# revision 18
# speedup vs baseline: 1.1198x; 1.1198x over previous
"""BiMamba layer on 8 TRN2 NeuronCores.

Sharding: 8 cores = 4 (dir,batch) pairs x 2 halves of d_inner; host flips
the sequence for the backward direction, transposes to [channel, token]
layout, and sums the 4 partial outputs per batch + residual at the end.

Per-core pipeline (all on-chip, [channel, token] layout):
  LN (stats via ones-matmul, rstd = Exp(-0.5*Ln(var+eps)))
  -> in_proj (f32r matmuls; LN gain/bias folded into weights on host)
  -> depthwise causal conv (tensor_scalar + shifted scalar_tensor_tensor)
  -> silu via Sigmoid+mult -> xproj/dtproj (bf16 matmuls)
  -> softplus = Ln(1+Exp(x)) (same ACT table as the scan exps)
  -> selective scan: A[d,n] = -n  =>  dA_n = Exp(-n*dt) with immediate ACT
     scale; recurrence via DVE tensor_tensor_scan per n; y = sum_n C_n*h_n
  -> y = (D*u + ysc) * silu(z) -> fused (out_proj @ fuse) matmul (f32r).

Channel permutation on host makes the SPMD program uniform: each core's
512 scan channels are always xc rows 0..511.
"""
import sys
sys.path.insert(0, '/opt/trn_rl_repo')
import numpy as np
import ml_dtypes
from contextlib import ExitStack

import concourse.bass as bass
import concourse.tile as tile
from concourse import bacc, mybir
from concourse.bass_utils import run_bass_kernel_spmd

AF = mybir.ActivationFunctionType
OP = mybir.AluOpType
F32, BF16, F32R = mybir.dt.float32, mybir.dt.bfloat16, mybir.dt.float32r
BF = ml_dtypes.bfloat16

DIM, DSTATE, DCONV, DINNER, DTRANK, B, L = 512, 16, 4, 1024, 32, 2, 2048
HALF = DINNER // 2
P = 128
NT = L // 512
KD = DIM // P               # 4 k-tiles over D
MI = (DINNER + HALF) // P   # 12 in_proj M-tiles (8 xc + 4 z)
MX = DINNER // P            # 8 xc tiles
MH = HALF // P              # 4 scan-channel tiles
EPS = 1e-5

_CACHE = {}


def _build(trace_sim=False):
    nc = bacc.Bacc("TRN2", target_bir_lowering=False, debug=False,
                   num_devices=8)
    dram = {}
    def din(name, shape, dt):
        dram[name] = nc.dram_tensor(name, shape, dt, kind="ExternalInput").ap()
    din("xT", [DIM, L], F32R)
    din("inw", [DIM, P * MI], F32R)
    din("inb", [P, MI], F32)
    din("convw", [DINNER, DCONV], F32)
    din("convb", [DINNER, 1], F32)
    din("xpw", [DINNER, 64], BF16)
    din("dtpw", [DTRANK, HALF], BF16)
    din("dtb", [P, MH], F32)
    din("dvec", [P, MH], F32)
    din("weff", [HALF, DIM], BF16)
    din("ones", [P, 1], F32R)
    din("ident", [P, P], BF16)
    outT = nc.dram_tensor("outT", [DIM, L], F32, kind="ExternalOutput").ap()
    bcrows = nc.dram_tensor("bcrows", [2 * DSTATE, L], BF16).ap()
    stats = nc.dram_tensor("stats", [2, L], BF16).ap()

    with tile.TileContext(nc, trace_sim=trace_sim) as tc, ExitStack() as ctx:
        sb = ctx.enter_context(tc.tile_pool(name="sb", bufs=1))
        tf = ctx.enter_context(tc.tile_pool(name="tf", bufs=2))
        ppA = tc.alloc_tile_pool(name="ppA", bufs=2, space="PSUM")

        # ---- weights ----
        inw = [sb.tile([P, P * MI], F32R, tag=f"w{k}", name=f"w{k}") for k in range(KD)]
        for k in range(KD):
            nc.sync.dma_start(inw[k][:], dram["inw"][k * P:(k + 1) * P, :])
        inb = sb.tile([P, MI], F32, tag="inb", name="inb")
        nc.sync.dma_start(inb[:], dram["inb"][:])
        convw = [sb.tile([P, DCONV], F32, tag=f"cw{j}", name=f"cw{j}") for j in range(MX)]
        convb = [sb.tile([P, 1], F32, tag=f"cb{j}", name=f"cb{j}") for j in range(MX)]
        for j in range(MX):
            nc.sync.dma_start(convw[j][:], dram["convw"][j * P:(j + 1) * P, :])
            nc.sync.dma_start(convb[j][:], dram["convb"][j * P:(j + 1) * P, :])
        xpw = [sb.tile([P, 64], BF16, tag=f"xpw{k}", name=f"xpw{k}") for k in range(MX)]
        for k in range(MX):
            nc.sync.dma_start(xpw[k][:], dram["xpw"][k * P:(k + 1) * P, :])
        dtpw = sb.tile([DTRANK, HALF], BF16, tag="dtpw", name="dtpw")
        nc.sync.dma_start(dtpw[:], dram["dtpw"][:])
        dtb = sb.tile([P, MH], F32, tag="dtb", name="dtb")
        nc.sync.dma_start(dtb[:], dram["dtb"][:])
        dvec = sb.tile([P, MH], F32, tag="dvec", name="dvec")
        nc.sync.dma_start(dvec[:], dram["dvec"][:])
        weff = [sb.tile([P, DIM], BF16, tag=f"wef{k}", name=f"wef{k}") for k in range(MH)]
        for k in range(MH):
            nc.sync.dma_start(weff[k][:], dram["weff"][k * P:(k + 1) * P, :])
        ones = sb.tile([P, 1], F32R, tag="ones", name="ones")
        nc.sync.dma_start(ones[:], dram["ones"][:])
        ident = sb.tile([P, P], BF16, tag="ident", name="ident")
        nc.sync.dma_start(ident[:], dram["ident"][:])
        ceps = sb.tile([1, 1], F32, tag="ceps", name="ceps")
        nc.vector.memset(ceps[:], EPS)
        cone = sb.tile([P, 1], F32, tag="cone", name="cone")
        nc.vector.memset(cone[:], 1.0)
        ones16 = sb.tile([P, 1], BF16, tag="ones16", name="ones16")
        nc.vector.memset(ones16[:], 1.0)

        # ---- phase A: LN stats on raw xT; mean/bias folded into in_proj via
        #      rank-1 matmul rows; rstd folded into the PSUM evacuation ----
        pssum = ppA.tile([1, L], F32, tag="pa", name="st0")
        pssq = ppA.tile([1, L], F32, tag="pa", name="st1")
        h0 = []
        for k in range(KD):
            xtk = sb.tile([P, L], F32R, tag=f"g{k}", name=f"g{k}")
            nc.sync.dma_start(xtk[:], dram["xT"][k * P:(k + 1) * P, :])
            h0.append(xtk)
            xsq = sb.tile([P, L], BF16, tag=f"m{k % 2}", name=f"m{k % 2}")
            nc.scalar.activation(xsq[:], xtk[:], AF.Square)
            for c in range(NT):
                sl = slice(c * 512, (c + 1) * 512)
                nc.tensor.matmul(pssum[:, sl], ones[:],
                                 xtk[:, sl],
                                 start=(k == 0), stop=(k == KD - 1))
                nc.tensor.matmul(pssq[:, sl], ones16[:],
                                 xsq[:, sl],
                                 start=(k == 0), stop=(k == KD - 1))
        mu = sb.tile([1, L], F32, tag="s0", name="mu")
        m2 = sb.tile([1, L], F32, tag="s1", name="m2")
        nc.scalar.activation(mu[:], pssum[:], AF.Copy, scale=1.0 / DIM)
        nc.scalar.activation(m2[:], pssq[:], AF.Copy, scale=1.0 / DIM)
        mu2 = ppA.tile([1, L], F32, tag="pa", name="mu2")
        nc.vector.tensor_tensor(mu2[:], mu[:], mu[:], OP.mult)
        var = ppA.tile([1, L], F32, tag="pa", name="var")
        nc.vector.tensor_tensor(var[:], m2[:], mu2[:], OP.subtract)
        lnv = ppA.tile([1, L], F32, tag="pa", name="lnv")
        nc.scalar.activation(lnv[:], var[:], AF.Ln, bias=ceps[:])
        mu16 = sb.tile([1, L], BF16, tag="s1", name="mu16")
        nc.scalar.activation(mu16[:], mu[:], AF.Copy)
        nc.sync.dma_start(stats[0:1, :], mu16[:])
        rstd16 = sb.tile([1, L], BF16, tag="s0", name="rstd16")
        nc.scalar.activation(rstd16[:], lnv[:], AF.Exp, scale=-0.5)
        nc.sync.dma_start(stats[1:2, :], rstd16[:])
        mu_b = sb.tile([P, L], BF16, tag="m0", name="m0")
        rstd_b = sb.tile([P, L], BF16, tag="m1", name="m1")
        nc.sync.dma_start(mu_b[:], stats[0:1, :].broadcast_to([P, L]))
        nc.sync.dma_start(rstd_b[:], stats[1:2, :].broadcast_to([P, L]))
        for k in range(KD):
            nc.vector.tensor_tensor(h0[k][:], h0[k][:], mu_b[:], OP.subtract)
            nc.vector.tensor_tensor(h0[k][:], h0[k][:], rstd_b[:], OP.mult)

        # ---- phase B: in_proj (raw x; rank-1 rows fold LN mean + bias) ----
        ppA.release()
        ppB = tc.alloc_tile_pool(name="ppB", bufs=4, space="PSUM")
        ppC = tc.alloc_tile_pool(name="ppC", bufs=1, space="PSUM")
        xc = [sb.tile([P, L], BF16, tag=f"x{j}", name=f"x{j}") for j in range(MX)]
        z = [sb.tile([P, L], BF16, tag=f"z{j}", name=f"z{j}") for j in range(MI - MX)]
        for m in range(MI):
            dest = xc[m] if m < MX else z[m - MX]
            for c in range(NT):
                sl = slice(c * 512, (c + 1) * 512)
                pmm = ppB.tile([P, 512], F32, tag="mmb", name="mmb")
                for k in range(KD):
                    nc.tensor.matmul(pmm[:], inw[k][:, m * P:(m + 1) * P],
                                     h0[k][:, sl],
                                     start=(k == 0), stop=(k == KD - 1))
                if (m * NT + c) % 2 == 0:
                    nc.scalar.activation(dest[:, sl], pmm[:], AF.Identity,
                                         bias=inb[:, m:m + 1])
                else:
                    nc.vector.tensor_scalar_add(dest[:, sl], pmm[:], inb[:, m:m + 1])

        # ---- phase C: depthwise conv on PE (diag-weight matmuls, shifted
        #      rhs; bias via rank-1); silu straight off PSUM -> u ----
        u = []
        for j in range(MX):
            dj = []
            for k in range(DCONV):
                dk = sb.tile([P, P], BF16, tag=f"dw{k}_{j % 2}", name=f"dw{k}_{j % 2}")
                nc.vector.tensor_scalar_mul(dk[:], ident[:], convw[j][:, k:k + 1])
                dj.append(dk)
            uj = sb.tile([P, L], BF16, tag=f"x{j}", name=f"x{j}")
            for c in range(NT):
                sl = slice(c * 512, (c + 1) * 512)
                pcv = ppB.tile([P, 512], F32, tag="mmb", name="cv")
                nc.tensor.matmul(pcv[:], dj[3][:], xc[j][:, sl],
                                 start=True, stop=False)
                for k in range(3):
                    off = 3 - k
                    lo = c * 512 - off
                    if lo < 0:
                        nc.tensor.matmul(pcv[:, off:], dj[k][:],
                                         xc[j][:, 0:512 - off],
                                         start=False, stop=(k == 2))
                    else:
                        nc.tensor.matmul(pcv[:], dj[k][:],
                                         xc[j][:, lo:lo + 512],
                                         start=False, stop=(k == 2))
                sg = tf.tile([P, 512], BF16, tag="tf", name="sg")
                nc.scalar.activation(sg[:], pcv[:], AF.Sigmoid,
                                     bias=convb[j][:])
                nc.vector.scalar_tensor_tensor(uj[:, sl], pcv[:], convb[j][:],
                                               sg[:], OP.add, OP.mult)
            u.append(uj)

        # ---- phase D: xproj, dtproj, softplus, dtu, D*u, silu(z) ----
        px = ppC.tile([64, L], F32, tag="big", name="px")
        for c in range(NT):
            sl = slice(c * 512, (c + 1) * 512)
            for k in range(MX):
                nc.tensor.matmul(px[:, sl], xpw[k][:], u[k][:, sl],
                                 start=(k == 0), stop=(k == MX - 1))
        dbl = sb.tile([64, L], BF16, tag="dbl", name="dbl")
        for c in range(NT):
            sl = slice(c * 512, (c + 1) * 512)
            nc.scalar.activation(dbl[:, sl], px[:, sl], AF.Copy)
        nc.sync.dma_start(bcrows[:], dbl[DTRANK:64, :])

        dt = []
        dtu = []
        yD = []
        for m in range(MH):
            pd = ppC.tile([P, L], F32, tag="big", name="pd")
            for c in range(NT):
                sl = slice(c * 512, (c + 1) * 512)
                nc.tensor.matmul(pd[:, sl], dtpw[:, m * P:(m + 1) * P],
                                 dbl[0:DTRANK, sl], start=True, stop=True)
            dtm = sb.tile([P, L], BF16, tag=f"f{m}", name=f"f{m}")
            nc.scalar.activation(dtm[:], pd[:], AF.Exp, bias=dtb[:, m:m + 1])
            nc.scalar.activation(dtm[:], dtm[:], AF.Ln, bias=cone[:])
            dt.append(dtm)
            dtum = sb.tile([P, L], BF16, tag=("m0", "m1", "d2", "d3")[m], name="dtu")
            nc.vector.tensor_tensor(dtum[:], dtm[:], u[m][:], OP.mult)
            dtu.append(dtum)
            yDm = sb.tile([P, L], BF16, tag=f"yd{m}", name=f"yd{m}")
            nc.vector.tensor_scalar_mul(yDm[:], u[m][:], dvec[:, m:m + 1])
            yD.append(yDm)
        zs = []
        for j in range(MH):
            sgz = tf.tile([P, L], BF16, tag="tf", name="tf")
            nc.scalar.activation(sgz[:], z[j][:], AF.Sigmoid)
            zsj = sb.tile([P, L], BF16, tag=f"x{MH + j}", name=f"x{MH + j}")
            nc.vector.tensor_tensor(zsj[:], z[j][:], sgz[:], OP.mult)
            zs.append(zsj)

        # ---- phase E: 16 scan passes; ysc accumulated in PSUM via
        #      identity-matmul on PE (f32 accumulation, no DVE adds) ----
        ppC.release()
        ppB.release()
        ppE = tc.alloc_tile_pool(name="ppE", bufs=2, space="PSUM")
        y = []
        for mhalf in range(2):
            yp = [ppE.tile([P, L], F32, tag="ysp", name="ysp") for _ in range(2)]
            for n in range(1, DSTATE + 1):
                bn = sb.tile([P, L], BF16, tag=f"x{n % 2}", name="bn")
                cn = sb.tile([P, L], BF16, tag=f"x{2 + n % 2}", name="cn")
                nc.sync.dma_start(bn[:], bcrows[n - 1:n, :].broadcast_to([P, L]))
                nc.sync.dma_start(cn[:], bcrows[DSTATE + n - 1:DSTATE + n, :]
                                  .broadcast_to([P, L]))
                for j in range(2):
                    m = mhalf * 2 + j
                    it = n * 2 + j
                    dA = sb.tile([P, L], BF16, tag=f"w{it % 2}", name="dA")
                    nc.scalar.activation(dA[:], dt[m][:], AF.Exp, scale=float(-n))
                    dBu = sb.tile([P, L], BF16, tag=f"w{2 + it % 2}", name="dBu")
                    nc.vector.tensor_tensor(dBu[:], dtu[m][:], bn[:], OP.mult)
                    h = sb.tile([P, L], BF16, tag=f"z{it % 2}", name="h")
                    nc.vector.tensor_tensor_scan(h[:], dA[:], dBu[:], 0.0,
                                                 OP.mult, OP.add)
                    hc = sb.tile([P, L], BF16, tag=f"z{2 + it % 2}", name="hc")
                    hc_eng = nc.gpsimd if n % 2 == 0 else nc.vector
                    hc_eng.tensor_tensor(hc[:], h[:], cn[:], OP.mult)
                    for c in range(NT):
                        sl = slice(c * 512, (c + 1) * 512)
                        nc.tensor.matmul(yp[j][:, sl], ident[:], hc[:, sl],
                                         start=(n == 1), stop=(n == DSTATE))
            for j in range(2):
                m = mhalf * 2 + j
                ym = sb.tile([P, L], BF16, tag=f"g{m}", name=f"g{m}")
                nc.vector.tensor_tensor(ym[:], yD[m][:], yp[j][:], OP.add)
                nc.vector.tensor_tensor(ym[:], ym[:], zs[m][:], OP.mult)
                y.append(ym)

        # ---- phase F: y = (D*u + ysc) * zs; out_proj ----
        ppE.release()
        ppF = tc.alloc_tile_pool(name="ppF", bufs=2, space="PSUM")
        for half in range(2):
            po = [ppF.tile([P, L], F32, tag="po", name="po") for _ in range(2)]
            for k in range(MH):
                for j in range(2):
                    mo = half * 2 + j
                    for c in range(NT):
                        sl = slice(c * 512, (c + 1) * 512)
                        nc.tensor.matmul(po[j][:, sl],
                                         weff[k][:, mo * P:(mo + 1) * P],
                                         y[k][:, sl],
                                         start=(k == 0), stop=(k == MH - 1))
            for j in range(2):
                mo = half * 2 + j
                for c in range(NT):
                    sl = slice(c * 512, (c + 1) * 512)
                    ev = tf.tile([P, 512], F32, tag="tf", name="ev")
                    if (j * NT + c) % 2 == 0:
                        nc.scalar.activation(ev[:], po[j][:, sl], AF.Copy)
                    else:
                        nc.vector.tensor_copy(ev[:], po[j][:, sl])
                    nc.sync.dma_start(outT[mo * P:(mo + 1) * P, sl], ev[:])
        ppF.release()
    nc.compile()
    return nc


def _host_prep(inputs):
    f32 = np.float32
    x = np.asarray(inputs["x"], f32)
    ln_g = np.asarray(inputs["ln_g"], f32); ln_b = np.asarray(inputs["ln_b"], f32)
    in_w = np.asarray(inputs["in_w"], f32)
    conv_w = np.asarray(inputs["conv_w"], f32); conv_b = np.asarray(inputs["conv_b"], f32)
    xproj_w = np.asarray(inputs["xproj_w"], f32); dtproj_w = np.asarray(inputs["dtproj_w"], f32)
    dt_bias = np.asarray(inputs["dt_bias"], f32)
    D = np.asarray(inputs["D"], f32)
    out_w = np.asarray(inputs["out_w"], f32)
    fuse_w = np.asarray(inputs["fuse_w"], f32)

    maps = []
    for p in range(4):
        dir_, b = p // 2, p % 2
        W = in_w[dir_] * ln_g[None, :]          # [2*Di, D], LN gain folded
        in_bias_full = in_w[dir_] @ ln_b        # LN bias folded
        Weff_out = fuse_w[:, dir_ * DIM:(dir_ + 1) * DIM] @ out_w[dir_]
        xb = x[b] if dir_ == 0 else x[b, ::-1]
        for half in range(2):
            sl = slice(half * HALF, (half + 1) * HALF)
            # permute xc channels so this core's scan channels are rows 0..511
            perm = np.concatenate([np.arange(half * HALF, (half + 1) * HALF),
                                   np.arange((1 - half) * HALF, (2 - half) * HALF)])
            rows = np.concatenate([perm, DINNER + np.arange(half * HALF, (half + 1) * HALF)])
            m = dict(
                xT=np.ascontiguousarray(xb.T),
                inw=np.ascontiguousarray(W[rows].T),
                inb=np.ascontiguousarray(in_bias_full[rows].reshape(MI, P).T),
                convw=np.ascontiguousarray(conv_w[dir_][perm]),
                convb=np.ascontiguousarray(conv_b[dir_][perm].reshape(DINNER, 1)),
                xpw=np.ascontiguousarray(xproj_w[dir_][:, perm].T.astype(BF)),
                dtpw=np.ascontiguousarray(dtproj_w[dir_, sl].T.astype(BF)),
                dtb=np.ascontiguousarray(dt_bias[dir_, sl].reshape(MH, P).T),
                dvec=np.ascontiguousarray(D[dir_, sl].reshape(MH, P).T),
                weff=np.ascontiguousarray(Weff_out[:, sl].T.astype(BF)),
                ones=np.ones((P, 1), np.float32),
                ident=np.eye(P, dtype=np.float32).astype(BF),
            )
            maps.append(m)
    return maps


def kernel(**inputs):
    if "nc" not in _CACHE:
        _CACHE["nc"] = _build()
    nc = _CACHE["nc"]
    maps = _host_prep(inputs)
    res = run_bass_kernel_spmd(nc, maps, list(range(8)))
    x = np.asarray(inputs["x"], np.float32)
    fuse_b = np.asarray(inputs["fuse_b"], np.float32)
    out = x + fuse_b[None, None, :]
    for p in range(4):
        dir_, b = p // 2, p % 2
        for half in range(2):
            pt = np.asarray(res.results[p * 2 + half]["outT"], np.float32).T
            if dir_ == 1:
                pt = pt[::-1]
            out[b] += pt
    return out.astype(np.float32)

